# revision 63
# baseline (speedup 1.0000x reference)
"""Trainium2 Bass kernel for the k-mer transformer problem.

Semantics (k=3, one-hot 3-mer filters over 4 bases):
    z[l, c] = relu(x[l,d0] + x[l+1,d1] + x[l+2,d2] - 2)
      where c = 16*d0 + 4*d1 + d2,  l in [0, 99999)
    out[b, 0, r*33333 + q, c] = z[3q + r, c]      (mod-3 interleave)

Strategy: pure data parallel (batch elem b -> NeuronCore b), and the conv
is a matmul on the tensor engine (the only engine with headroom: vector
f32 adds cap at ~123 G elem/s, Pool's software tensor_tensor at ~40-65).

Key layout trick: within one phase r the positions l = 3q + r are stride-3,
so each output position consumes 12 *consecutive* x floats
x.flat[12q+4r : 12q+4r+12] -- no input replication. Two position streams
(A: q in [0,16704), B: q+16704) are packed as M=128 output rows
(64 channels x 2 streams), so the PE streams 2 positions per column.
The staged moving tensor holds one 24-row window per phase: row 8t+s of
phase r's window is x.flat[12m + 4(r+t) + s] for s<4 (stream A) and
x.flat[12(m+16704) + 4(r+t) + s-4] for s>=4 (stream B). Matmul operands
(both lhsT and rhs, which must share a base) must sit at SBUF base
partition 0/32/64, so the windows live at: xt0 [24 rows] with phase 0 at
rows 0:24, and xt12 [56 rows] with phase 1 at rows 0:24 and phase 2 at
rows 32:56. Phase 0's matmuls depend only on the small xt0 load (~1MB),
so the PE starts ~9us earlier than with a combined tile; xt12 streams in
under phase-0 compute. The stationary W[24,128] rides in the first 128
elements of every band's row segment 0 (one copy per base partition). W
is phase-invariant; the -2 bias folds into the relu evict, not the
matmul.

PSUM [128, 512] f32 banks (one matmul each, 2 banks per group, 4 groups
in flight so the PE never stalls) are evicted as relu(v-2) -> bf16 by ACT
and DVE in parallel column slices sized to measured rates (GPSIMD cannot
read PSUM). Output rides to HBM as bf16 (12.8 MB/core; harness tolerance
2e-2 dwarfs the ~6e-3 bf16 path error); the host transposes
[2*64, 16704] -> [q, c] and upcasts during the gather.

DMA budget: 2 loads + 2 stores x 3 phases = 8 <= 8 HWDGE sem lanes (9+
adds a lane-reuse wait -> walrus "Too many sync wait commands").

Measured on 8xTRN2 (neuron-profile): 114.0us baseline vector-engine
version -> 74.0us this version (PE busy 43us at an observed sustained
~1.2GHz, 1 col/cycle; loads ~14us; stores+drain tail ~17us).
"""

import sys

import ml_dtypes
import numpy as np

sys.path.insert(0, "/opt/trn_rl_repo")

import concourse.bacc as bacc  # noqa: E402
import concourse.mybir as mybir  # noqa: E402
from concourse.bass_utils import run_bass_kernel_spmd  # noqa: E402
from concourse.tile import TileContext  # noqa: E402

P = 128
Q = 33333  # valid q-positions per phase (99999 / 3)
M = 16704  # columns per stream (2 streams: q and q + M; 2*M = 33408 >= Q)
NROW = 24  # XT rows: 3 t-blocks x (4 A-rows + 4 B-rows), one window/phase
GROUP = 1024  # psum group columns (2 banks x 512); 4 groups in flight
# evict column split per group, proportional to measured engine rates
# (GPSIMD/Pool cannot read PSUM; ACT ~1.03 GHz and DVE ~0.89 GHz effective)
ACT_COLS = 608  # DVE gets the rest (416)
STORE_A = 10 * GROUP  # uneven store split so the trailing store is short
# Input loads run at half DMA-engine rate for descriptors above ~8KB, and
# in-place splits coalesce back into one big descriptor. So each staged row
# is six 3200-element segments separated by 64-element SBUF gaps
# (non-adjacent -> no coalescing; ~6.4KB descriptors run at full rate).
# Segment k holds data columns [3072k, 3072(k+1)) (last: ..16704), with W
# in the first 128 elements of segment 0 (data starts at offset 128 there).
# 4096 is a psum-group multiple, so no matmul slice crosses a segment
# boundary. Both loads ride the SP HWDGE ring: measured alternatives were
# slower (SWDGE/GPSIMD loads: +8us; xtb on the ACT ring: +19us, it stalls
# the ACT evict stream).
SEGDATA = 4096
SEG = 4416
SEGSTRIDE = SEG + 64  # 4480
NSEG = 4
ROWW = NSEG * SEGSTRIDE  # 17920 elements per staged SBUF row


def _phys(d):
    """Physical column of logical data column d."""
    k = min(d // SEGDATA, NSEG - 1)
    return k * SEGSTRIDE + (128 if k == 0 else 0) + (d - SEGDATA * k)
L = 100001
N_CORES = 8

_CACHE = {}


def _kmer_w():
    """Stationary [24, 128] weights: row 8*jj+s, s<4 -> tap (jj, d=s) of
    stream A (cols 0:64), s>=4 -> tap (jj, d=s-4) of stream B (cols 64:128).
    """
    w = np.zeros((24, 128), dtype=np.float32)
    c = np.arange(64)
    digits = np.stack([c // 16, (c // 4) % 4, c % 4])  # [jj, c]
    for k in range(24):
        jj, s = k // 8, k % 8
        blk, d = (0, s) if s < 4 else (1, s - 4)
        w[k, 64 * blk + c[digits[jj] == d]] = 1.0
    return w.astype(ml_dtypes.bfloat16)


def _build_bass():
    # Bacc (not raw Bass): its finalize() runs generate_event_semaphores,
    # which splits multi-sem waits (HW allows at most 1 wait per inst).
    nc = bacc.Bacc()
    f32 = mybir.dt.float32
    bf16 = mybir.dt.bfloat16
    add = mybir.AluOpType.add
    mx = mybir.AluOpType.max
    relu = mybir.ActivationFunctionType.Relu

    xt0a_d = nc.declare_dram_parameter("xt0a", [24, 2, SEG], bf16, isOutput=False)
    xt0b_d = nc.declare_dram_parameter("xt0b", [24, 2, SEG], bf16, isOutput=False)
    xt12_d = nc.declare_dram_parameter("xt12", [56, NSEG, SEG], bf16, isOutput=False)
    y_d = nc.declare_dram_parameter("y", [3, P, M], bf16, isOutput=True)

    with TileContext(nc) as tc:
        with (
            tc.tile_pool(name="xp", bufs=1) as xp,
            tc.tile_pool(name="pp", bufs=4, space="PSUM") as pp,
            tc.tile_pool(name="op_", bufs=2) as op_,
        ):
            # phase 0's band rides two half-tiles so the PE can start after
            # ~0.4MB of load; its matmuls for cols >= 8192 take lhsT from
            # xt0a (cross-tile, same base partition 0)
            xt0a_sb = xp.tile([24, 2 * SEGSTRIDE], bf16)
            nc.sync.dma_start(
                out=xt0a_sb.rearrange("p (c g) -> p c g", c=2)[:, :, 0:SEG],
                in_=xt0a_d[:],
            )
            xt0b_sb = xp.tile([24, 2 * SEGSTRIDE], bf16, tag="xt0b")
            nc.sync.dma_start(
                out=xt0b_sb.rearrange("p (c g) -> p c g", c=2)[:, :, 0:SEG],
                in_=xt0b_d[:],
            )
            xt12_sb = xp.tile([56, ROWW], bf16, tag="xt12")
            nc.sync.dma_start(
                out=xt12_sb.rearrange("p (c g) -> p c g", c=NSEG)[:, :, 0:SEG],
                in_=xt12_d[:],
            )
            bias_sb = xp.tile([P, 1], f32, tag="bias")
            nc.vector.memset(bias_sb, -2.0)
            for r in range(3):
                o = op_.tile([P, M], bf16, tag="o")
                # lhsT and rhs must share a base partition (0/32/64)
                if r == 0:
                    w_ap = xt0a_sb[0:24, 0:128]
                else:
                    w_ap = xt12_sb[32 * (r - 1) : 32 * (r - 1) + 24, 0:128]
                for g0 in range(0, M, GROUP):
                    gw = min(GROUP, M - g0)
                    ps = pp.tile([P, GROUP], f32, tag="ps")
                    for k0 in range(0, gw, 512):
                        kw = min(512, gw - k0)
                        d = g0 + k0
                        if r == 0:
                            if d < 2 * SEGDATA:
                                rhs = xt0a_sb[0:24]
                                c0 = _phys(d)
                            else:
                                rhs = xt0b_sb[0:24]
                                c0 = _phys(d) - 2 * SEGSTRIDE
                        else:
                            p0 = 32 * (r - 1)
                            rhs = xt12_sb[p0 : p0 + 24]
                            c0 = _phys(d)
                        nc.tensor.matmul(
                            out=ps[:, k0 : k0 + kw],
                            lhsT=w_ap,
                            rhs=rhs[:, c0 : c0 + kw],
                            start=True,
                            stop=True,
                        )
                    # relu(v - 2) -> bf16; ACT and DVE split the columns
                    sp = (ACT_COLS * gw) // GROUP
                    nc.scalar.activation(
                        o[:, g0 : g0 + sp],
                        ps[:, 0:sp],
                        relu,
                        bias=bias_sb,
                    )
                    nc.vector.tensor_scalar(
                        o[:, g0 + sp : g0 + gw],
                        ps[:, sp:gw],
                        -2.0,
                        0.0,
                        add,
                        mx,
                    )
                if r == 1:
                    # one full-phase store: pays for xt0's extra load DMA
                    # (8-lane budget); it drains fully under phase-2 compute
                    nc.sync.dma_start(out=y_d[1], in_=o[:])
                else:
                    nc.sync.dma_start(
                        out=y_d[r, :, 0:STORE_A], in_=o[:, 0:STORE_A]
                    )
                    nc.sync.dma_start(
                        out=y_d[r, :, STORE_A:M], in_=o[:, STORE_A:M]
                    )
    return nc


def _stage_inputs(x):
    """x: [8,1,L,4] f32 -> per-core {'xta': [56, M+128], 'xtb': [24, M]}."""
    w = _kmer_w()
    need = 12 * (2 * M - 1) + 28  # last col of the r=2 window reads up to here
    in_maps = []
    for b in range(x.shape[0]):
        xf = np.zeros(need, dtype=np.float32)
        xf[: L * 4] = x[b, 0].ravel()

        def band(r):
            out = np.empty((NROW, M), dtype=np.float32)
            for t in range(3):
                for s in range(4):
                    out[8 * t + s] = xf[4 * (r + t) + s :: 12][:M]
                    out[8 * t + s + 4] = xf[12 * M + 4 * (r + t) + s :: 12][:M]
            return out

        def segs(rows, data, wmat):
            """Pack data into NSEG padded segments (W leads segment 0)."""
            out = np.zeros((rows, NSEG, SEG), dtype=ml_dtypes.bfloat16)
            out[: wmat.shape[0], 0, 0:128] = wmat
            for k in range(NSEG):
                hi = SEGDATA * (k + 1) if k < NSEG - 1 else M
                chunk = data[:, SEGDATA * k : hi]
                off = 128 if k == 0 else 0
                out[: data.shape[0], k, off : off + chunk.shape[1]] = chunk
            return out

        b16 = ml_dtypes.bfloat16
        xt0 = segs(24, band(0).astype(b16), w)
        xt12 = np.zeros((56, NSEG, SEG), dtype=b16)
        xt12[0:24] = segs(24, band(1).astype(b16), w)
        xt12[32:56] = segs(24, band(2).astype(b16), w)
        in_maps.append(
            {
                "xt0a": np.ascontiguousarray(xt0[:, 0:2]),
                "xt0b": np.ascontiguousarray(xt0[:, 2:4]),
                "xt12": xt12,
            }
        )
    return in_maps


def _gather_output(results):
    out = np.empty((len(results), 1, 3 * Q, 64), dtype=np.float32)
    for b, res in enumerate(results):
        y = np.asarray(res["y"]).astype(np.float32)  # [3, 128, M]
        for r in range(3):
            zr = y[r].reshape(2, 64, M).transpose(0, 2, 1).reshape(2 * M, 64)
            out[b, 0, r * Q : (r + 1) * Q, :] = zr[:Q]
    return out


def _built_and_finalized():
    if "nc" not in _CACHE:
        nc = _build_bass()
        # run_bass_via_pjrt never finalizes; Bacc.finalize runs the register
        # allocation + sync-wait legalization passes walrus requires.
        nc.finalize()
        _CACHE["nc"] = nc
    return _CACHE["nc"]


def run(x, trace=False):
    nc = _built_and_finalized()
    in_maps = _stage_inputs(np.asarray(x, dtype=np.float32))
    bkr = run_bass_kernel_spmd(nc, in_maps, list(range(N_CORES)), trace=trace)
    return _gather_output(bkr.results), bkr


def kernel(x, W=None):
    out, _ = run(x, trace=False)
    return out


# revision 65
# speedup vs baseline: 1.0315x; 1.0315x over previous
"""Trainium2 Bass kernel for the k-mer transformer problem.

Semantics (k=3, one-hot 3-mer filters over 4 bases):
    z[l, c] = relu(x[l,d0] + x[l+1,d1] + x[l+2,d2] - 2)
      where c = 16*d0 + 4*d1 + d2,  l in [0, 99999)
    out[b, 0, r*33333 + q, c] = z[3q + r, c]      (mod-3 interleave)

Strategy: pure data parallel (batch elem b -> NeuronCore b), and the conv
is a matmul on the tensor engine (the only engine with headroom: vector
f32 adds cap at ~123 G elem/s, Pool's software tensor_tensor at ~40-65).

Key layout trick: within one phase r the positions l = 3q + r are stride-3,
so each output position consumes 12 *consecutive* x floats
x.flat[12q+4r : 12q+4r+12] -- no input replication. Two position streams
(A: q in [0,16704), B: q+16704) are packed as M=128 output rows
(64 channels x 2 streams), so the PE streams 2 positions per column.
The staged moving tensor holds one 24-row window per phase: row 8t+s of
phase r's window is x.flat[12m + 4(r+t) + s] for s<4 (stream A) and
x.flat[12(m+16704) + 4(r+t) + s-4] for s>=4 (stream B). Matmul operands
(both lhsT and rhs, which must share a base) must sit at SBUF base
partition 0/32/64, so the windows live at: xt0 [24 rows] with phase 0 at
rows 0:24, and xt12 [56 rows] with phase 1 at rows 0:24 and phase 2 at
rows 32:56. Phase 0's matmuls depend only on the small xt0 load (~1MB),
so the PE starts ~9us earlier than with a combined tile; xt12 streams in
under phase-0 compute. The stationary W[24,128] rides in the first 128
elements of every band's row segment 0 (one copy per base partition). W
is phase-invariant; the -2 bias folds into the relu evict, not the
matmul.

PSUM [128, 512] f32 banks (one matmul each, 2 banks per group, 4 groups
in flight so the PE never stalls) are evicted as relu(v-2) -> bf16 by ACT
and DVE in parallel column slices sized to measured rates (GPSIMD cannot
read PSUM). Output rides to HBM as bf16 (12.8 MB/core; harness tolerance
2e-2 dwarfs the ~6e-3 bf16 path error); the host transposes
[2*64, 16704] -> [q, c] and upcasts during the gather.

DMA budget: 2 loads + 2 stores x 3 phases = 8 <= 8 HWDGE sem lanes (9+
adds a lane-reuse wait -> walrus "Too many sync wait commands").

Measured on 8xTRN2 (neuron-profile): 114.0us baseline vector-engine
version -> 69.0us this version (PE starts at ~13us after the small xt0a
load and runs 44us busy at an observed sustained ~1.2GHz, 1 col/cycle;
stores + a ~6.5us semaphore-drain epilogue make up the tail).
"""

import sys

import ml_dtypes
import numpy as np

sys.path.insert(0, "/opt/trn_rl_repo")

import concourse.bacc as bacc  # noqa: E402
import concourse.mybir as mybir  # noqa: E402
from concourse.bass_utils import run_bass_kernel_spmd  # noqa: E402
from concourse.tile import TileContext  # noqa: E402

P = 128
Q = 33333  # valid q-positions per phase (99999 / 3)
M = 16704  # columns per stream (2 streams: q and q + M; 2*M = 33408 >= Q)
NROW = 24  # XT rows: 3 t-blocks x (4 A-rows + 4 B-rows), one window/phase
GROUP = 1024  # psum group columns (2 banks x 512); 4 groups in flight
# evict column split per group, proportional to measured engine rates
# (GPSIMD/Pool cannot read PSUM; ACT ~1.03 GHz and DVE ~0.89 GHz effective)
ACT_COLS = 608  # DVE gets the rest (416)
STORE_A = 10 * GROUP  # uneven store split so the trailing store is short
# Input loads run at half DMA-engine rate for descriptors above ~8KB, and
# in-place splits coalesce back into one big descriptor. So each staged row
# is six 3200-element segments separated by 64-element SBUF gaps
# (non-adjacent -> no coalescing; ~6.4KB descriptors run at full rate).
# Segment k holds data columns [3072k, 3072(k+1)) (last: ..16704), with W
# in the first 128 elements of segment 0 (data starts at offset 128 there).
# 4096 is a psum-group multiple, so no matmul slice crosses a segment
# boundary. Both loads ride the SP HWDGE ring: measured alternatives were
# slower (SWDGE/GPSIMD loads: +8us; xtb on the ACT ring: +19us, it stalls
# the ACT evict stream).
SEGDATA = 4096
SEG = 4416
SEGSTRIDE = SEG + 64  # 4480
NSEG = 4
ROWW = NSEG * SEGSTRIDE  # 17920 elements per staged SBUF row


def _phys(d):
    """Physical column of logical data column d."""
    k = min(d // SEGDATA, NSEG - 1)
    return k * SEGSTRIDE + (128 if k == 0 else 0) + (d - SEGDATA * k)
L = 100001
N_CORES = 8

_CACHE = {}


def _kmer_w():
    """Stationary [24, 128] weights: row 8*jj+s, s<4 -> tap (jj, d=s) of
    stream A (cols 0:64), s>=4 -> tap (jj, d=s-4) of stream B (cols 64:128).
    """
    w = np.zeros((24, 128), dtype=np.float32)
    c = np.arange(64)
    digits = np.stack([c // 16, (c // 4) % 4, c % 4])  # [jj, c]
    for k in range(24):
        jj, s = k // 8, k % 8
        blk, d = (0, s) if s < 4 else (1, s - 4)
        w[k, 64 * blk + c[digits[jj] == d]] = 1.0
    return w.astype(ml_dtypes.bfloat16)


def _build_bass():
    # Bacc (not raw Bass): its finalize() runs generate_event_semaphores,
    # which splits multi-sem waits (HW allows at most 1 wait per inst).
    nc = bacc.Bacc()
    f32 = mybir.dt.float32
    bf16 = mybir.dt.bfloat16
    add = mybir.AluOpType.add
    mx = mybir.AluOpType.max
    relu = mybir.ActivationFunctionType.Relu

    xt0a_d = nc.declare_dram_parameter("xt0a", [24, 2, SEG], bf16, isOutput=False)
    xt0b_d = nc.declare_dram_parameter("xt0b", [24, 2, SEG], bf16, isOutput=False)
    xt12_d = nc.declare_dram_parameter("xt12", [56, NSEG, SEG], bf16, isOutput=False)
    y_d = nc.declare_dram_parameter("y", [3, P, M], bf16, isOutput=True)

    with TileContext(nc) as tc:
        with (
            tc.tile_pool(name="xp", bufs=1) as xp,
            tc.tile_pool(name="pp", bufs=4, space="PSUM") as pp,
            tc.tile_pool(name="op_", bufs=2) as op_,
        ):
            # phase 0's band rides two half-tiles so the PE can start after
            # ~0.4MB of load; its matmuls for cols >= 8192 take lhsT from
            # xt0a (cross-tile, same base partition 0)
            xt0a_sb = xp.tile([24, 2 * SEGSTRIDE], bf16)
            nc.sync.dma_start(
                out=xt0a_sb.rearrange("p (c g) -> p c g", c=2)[:, :, 0:SEG],
                in_=xt0a_d[:],
            )
            xt0b_sb = xp.tile([24, 2 * SEGSTRIDE], bf16, tag="xt0b")
            nc.sync.dma_start(
                out=xt0b_sb.rearrange("p (c g) -> p c g", c=2)[:, :, 0:SEG],
                in_=xt0b_d[:],
            )
            xt12_sb = xp.tile([56, ROWW], bf16, tag="xt12")
            nc.sync.dma_start(
                out=xt12_sb.rearrange("p (c g) -> p c g", c=NSEG)[:, :, 0:SEG],
                in_=xt12_d[:],
            )
            bias_sb = xp.tile([P, 1], f32, tag="bias")
            nc.vector.memset(bias_sb, -2.0)
            for r in range(3):
                o = op_.tile([P, M], bf16, tag="o")
                # lhsT and rhs must share a base partition (0/32/64)
                if r == 0:
                    w_ap = xt0a_sb[0:24, 0:128]
                else:
                    w_ap = xt12_sb[32 * (r - 1) : 32 * (r - 1) + 24, 0:128]
                for g0 in range(0, M, GROUP):
                    gw = min(GROUP, M - g0)
                    ps = pp.tile([P, GROUP], f32, tag="ps")
                    for k0 in range(0, gw, 512):
                        kw = min(512, gw - k0)
                        d = g0 + k0
                        if r == 0:
                            if d < 2 * SEGDATA:
                                rhs = xt0a_sb[0:24]
                                c0 = _phys(d)
                            else:
                                rhs = xt0b_sb[0:24]
                                c0 = _phys(d) - 2 * SEGSTRIDE
                        else:
                            p0 = 32 * (r - 1)
                            rhs = xt12_sb[p0 : p0 + 24]
                            c0 = _phys(d)
                        nc.tensor.matmul(
                            out=ps[:, k0 : k0 + kw],
                            lhsT=w_ap,
                            rhs=rhs[:, c0 : c0 + kw],
                            start=True,
                            stop=True,
                        )
                    # relu(v - 2) -> bf16; ACT and DVE split the columns
                    sp = (ACT_COLS * gw) // GROUP
                    nc.scalar.activation(
                        o[:, g0 : g0 + sp],
                        ps[:, 0:sp],
                        relu,
                        bias=bias_sb,
                    )
                    nc.vector.tensor_scalar(
                        o[:, g0 + sp : g0 + gw],
                        ps[:, sp:gw],
                        -2.0,
                        0.0,
                        add,
                        mx,
                    )
                if r == 0:
                    # one full-phase store: pays for xt0's extra load DMA
                    # (8-lane budget) and overlaps phases 1-2 (phase-1 jumbo
                    # measured worse: it backs up the queue ahead of the
                    # phase-2 stores)
                    nc.sync.dma_start(out=y_d[0], in_=o[:])
                else:
                    nc.sync.dma_start(
                        out=y_d[r, :, 0:STORE_A], in_=o[:, 0:STORE_A]
                    )
                    nc.sync.dma_start(
                        out=y_d[r, :, STORE_A:M], in_=o[:, STORE_A:M]
                    )
    return nc


def _stage_inputs(x):
    """x: [8,1,L,4] f32 -> per-core {'xta': [56, M+128], 'xtb': [24, M]}."""
    w = _kmer_w()
    need = 12 * (2 * M - 1) + 28  # last col of the r=2 window reads up to here
    in_maps = []
    for b in range(x.shape[0]):
        xf = np.zeros(need, dtype=np.float32)
        xf[: L * 4] = x[b, 0].ravel()

        def band(r):
            out = np.empty((NROW, M), dtype=np.float32)
            for t in range(3):
                for s in range(4):
                    out[8 * t + s] = xf[4 * (r + t) + s :: 12][:M]
                    out[8 * t + s + 4] = xf[12 * M + 4 * (r + t) + s :: 12][:M]
            return out

        def segs(rows, data, wmat):
            """Pack data into NSEG padded segments (W leads segment 0)."""
            out = np.zeros((rows, NSEG, SEG), dtype=ml_dtypes.bfloat16)
            out[: wmat.shape[0], 0, 0:128] = wmat
            for k in range(NSEG):
                hi = SEGDATA * (k + 1) if k < NSEG - 1 else M
                chunk = data[:, SEGDATA * k : hi]
                off = 128 if k == 0 else 0
                out[: data.shape[0], k, off : off + chunk.shape[1]] = chunk
            return out

        b16 = ml_dtypes.bfloat16
        xt0 = segs(24, band(0).astype(b16), w)
        xt12 = np.zeros((56, NSEG, SEG), dtype=b16)
        xt12[0:24] = segs(24, band(1).astype(b16), w)
        xt12[32:56] = segs(24, band(2).astype(b16), w)
        in_maps.append(
            {
                "xt0a": np.ascontiguousarray(xt0[:, 0:2]),
                "xt0b": np.ascontiguousarray(xt0[:, 2:4]),
                "xt12": xt12,
            }
        )
    return in_maps


def _gather_output(results):
    out = np.empty((len(results), 1, 3 * Q, 64), dtype=np.float32)
    for b, res in enumerate(results):
        y = np.asarray(res["y"]).astype(np.float32)  # [3, 128, M]
        for r in range(3):
            zr = y[r].reshape(2, 64, M).transpose(0, 2, 1).reshape(2 * M, 64)
            out[b, 0, r * Q : (r + 1) * Q, :] = zr[:Q]
    return out


def _built_and_finalized():
    if "nc" not in _CACHE:
        nc = _build_bass()
        # run_bass_via_pjrt never finalizes; Bacc.finalize runs the register
        # allocation + sync-wait legalization passes walrus requires.
        nc.finalize()
        _CACHE["nc"] = nc
    return _CACHE["nc"]


def run(x, trace=False):
    nc = _built_and_finalized()
    in_maps = _stage_inputs(np.asarray(x, dtype=np.float32))
    bkr = run_bass_kernel_spmd(nc, in_maps, list(range(N_CORES)), trace=trace)
    return _gather_output(bkr.results), bkr


def kernel(x, W=None):
    out, _ = run(x, trace=False)
    return out


# revision 66
# speedup vs baseline: 1.0325x; 1.0010x over previous
"""Trainium2 Bass kernel for the k-mer transformer problem.

Semantics (k=3, one-hot 3-mer filters over 4 bases):
    z[l, c] = relu(x[l,d0] + x[l+1,d1] + x[l+2,d2] - 2)
      where c = 16*d0 + 4*d1 + d2,  l in [0, 99999)
    out[b, 0, r*33333 + q, c] = z[3q + r, c]      (mod-3 interleave)

Strategy: pure data parallel (batch elem b -> NeuronCore b), and the conv
is a matmul on the tensor engine (the only engine with headroom: vector
f32 adds cap at ~123 G elem/s, Pool's software tensor_tensor at ~40-65).

Key layout trick: within one phase r the positions l = 3q + r are stride-3,
so each output position consumes 12 *consecutive* x floats
x.flat[12q+4r : 12q+4r+12] -- no input replication. Two position streams
(A: q in [0,16704), B: q+16704) are packed as M=128 output rows
(64 channels x 2 streams), so the PE streams 2 positions per column.
The staged moving tensor holds one 24-row window per phase: row 8t+s of
phase r's window is x.flat[12m + 4(r+t) + s] for s<4 (stream A) and
x.flat[12(m+16704) + 4(r+t) + s-4] for s>=4 (stream B). Matmul operands
(both lhsT and rhs, which must share a base) must sit at SBUF base
partition 0/32/64, so the windows live at: xt0 [24 rows] with phase 0 at
rows 0:24, and xt12 [56 rows] with phase 1 at rows 0:24 and phase 2 at
rows 32:56. Phase 0's matmuls depend only on the small xt0 load (~1MB),
so the PE starts ~9us earlier than with a combined tile; xt12 streams in
under phase-0 compute. The stationary W[24,128] rides in the first 128
elements of every band's row segment 0 (one copy per base partition). W
is phase-invariant; the -2 bias folds into the relu evict, not the
matmul.

PSUM [128, 512] f32 banks (one matmul each, 2 banks per group, 4 groups
in flight so the PE never stalls) are evicted as relu(v-2) -> bf16 by ACT
and DVE in parallel column slices sized to measured rates (GPSIMD cannot
read PSUM). Output rides to HBM as bf16 (12.8 MB/core; harness tolerance
2e-2 dwarfs the ~6e-3 bf16 path error); the host transposes
[2*64, 16704] -> [q, c] and upcasts during the gather.

DMA budget: 2 loads + 2 stores x 3 phases = 8 <= 8 HWDGE sem lanes (9+
adds a lane-reuse wait -> walrus "Too many sync wait commands").

Measured on 8xTRN2 (neuron-profile): 114.0us baseline vector-engine
version -> 69.0us this version (PE starts at ~13us after the small xt0a
load and runs 44us busy at an observed sustained ~1.2GHz, 1 col/cycle;
stores + a ~6.5us semaphore-drain epilogue make up the tail).
"""

import sys

import ml_dtypes
import numpy as np

sys.path.insert(0, "/opt/trn_rl_repo")

import concourse.bacc as bacc  # noqa: E402
import concourse.mybir as mybir  # noqa: E402
from concourse.bass_utils import run_bass_kernel_spmd  # noqa: E402
from concourse.tile import TileContext  # noqa: E402

P = 128
Q = 33333  # valid q-positions per phase (99999 / 3)
M = 16704  # columns per stream (2 streams: q and q + M; 2*M = 33408 >= Q)
NROW = 24  # XT rows: 3 t-blocks x (4 A-rows + 4 B-rows), one window/phase
GROUP = 1024  # psum group columns (2 banks x 512); 4 groups in flight
# evict column split per group, proportional to measured engine rates
# (GPSIMD/Pool cannot read PSUM; ACT ~1.03 GHz and DVE ~0.89 GHz effective)
ACT_COLS = 608  # DVE gets the rest (416)
STORE_A = 10 * GROUP  # uneven store split so the trailing store is short
# Input loads run at half DMA-engine rate for descriptors above ~8KB, and
# in-place splits coalesce back into one big descriptor. So each staged row
# is six 3200-element segments separated by 64-element SBUF gaps
# (non-adjacent -> no coalescing; ~6.4KB descriptors run at full rate).
# Segment k holds data columns [3072k, 3072(k+1)) (last: ..16704), with W
# in the first 128 elements of segment 0 (data starts at offset 128 there).
# 4096 is a psum-group multiple, so no matmul slice crosses a segment
# boundary. Both loads ride the SP HWDGE ring: measured alternatives were
# slower (SWDGE/GPSIMD loads: +8us; xtb on the ACT ring: +19us, it stalls
# the ACT evict stream).
SEGDATA = 4096
SEG = 4416
SEGSTRIDE = SEG + 64  # 4480
NSEG = 4
ROWW = NSEG * SEGSTRIDE  # 17920 elements per staged SBUF row


def _phys(d):
    """Physical column of logical data column d."""
    k = min(d // SEGDATA, NSEG - 1)
    return k * SEGSTRIDE + (128 if k == 0 else 0) + (d - SEGDATA * k)
L = 100001
N_CORES = 8

_CACHE = {}


def _kmer_w():
    """Stationary [24, 128] weights: row 8*jj+s, s<4 -> tap (jj, d=s) of
    stream A (cols 0:64), s>=4 -> tap (jj, d=s-4) of stream B (cols 64:128).
    """
    w = np.zeros((24, 128), dtype=np.float32)
    c = np.arange(64)
    digits = np.stack([c // 16, (c // 4) % 4, c % 4])  # [jj, c]
    for k in range(24):
        jj, s = k // 8, k % 8
        blk, d = (0, s) if s < 4 else (1, s - 4)
        w[k, 64 * blk + c[digits[jj] == d]] = 1.0
    return w.astype(ml_dtypes.bfloat16)


def _build_bass():
    # Bacc (not raw Bass): its finalize() runs generate_event_semaphores,
    # which splits multi-sem waits (HW allows at most 1 wait per inst).
    nc = bacc.Bacc()
    f32 = mybir.dt.float32
    bf16 = mybir.dt.bfloat16
    add = mybir.AluOpType.add
    mx = mybir.AluOpType.max
    relu = mybir.ActivationFunctionType.Relu

    xt0a_d = nc.declare_dram_parameter("xt0a", [24, 2, SEG], bf16, isOutput=False)
    xt0b_d = nc.declare_dram_parameter("xt0b", [24, 2, SEG], bf16, isOutput=False)
    xt12_d = nc.declare_dram_parameter("xt12", [56, NSEG, SEG], bf16, isOutput=False)
    y_d = nc.declare_dram_parameter("y", [3, P, M], bf16, isOutput=True)

    with TileContext(nc) as tc:
        with (
            tc.tile_pool(name="xp", bufs=1) as xp,
            tc.tile_pool(name="pp", bufs=4, space="PSUM") as pp,
            tc.tile_pool(name="op_", bufs=2) as op_,
        ):
            # phase 0's band rides two half-tiles so the PE can start after
            # ~0.4MB of load; its matmuls for cols >= 8192 take lhsT from
            # xt0a (cross-tile, same base partition 0)
            xt0a_sb = xp.tile([24, 2 * SEGSTRIDE], bf16)
            nc.sync.dma_start(
                out=xt0a_sb.rearrange("p (c g) -> p c g", c=2)[:, :, 0:SEG],
                in_=xt0a_d[:],
            )
            xt0b_sb = xp.tile([24, 2 * SEGSTRIDE], bf16, tag="xt0b")
            nc.sync.dma_start(
                out=xt0b_sb.rearrange("p (c g) -> p c g", c=2)[:, :, 0:SEG],
                in_=xt0b_d[:],
            )
            xt12_sb = xp.tile([56, ROWW], bf16, tag="xt12")
            nc.sync.dma_start(
                out=xt12_sb.rearrange("p (c g) -> p c g", c=NSEG)[:, :, 0:SEG],
                in_=xt12_d[:],
            )
            bias_sb = xp.tile([P, 1], f32, tag="bias")
            nc.vector.memset(bias_sb, -2.0)
            for r in range(3):
                o = op_.tile([P, M], bf16, tag="o")
                # lhsT and rhs must share a base partition (0/32/64)
                if r == 0:
                    w_ap = xt0a_sb[0:24, 0:128]
                else:
                    w_ap = xt12_sb[32 * (r - 1) : 32 * (r - 1) + 24, 0:128]
                for g0 in range(0, M, GROUP):
                    gw = min(GROUP, M - g0)
                    ps = pp.tile([P, GROUP], f32, tag="ps")
                    for k0 in range(0, gw, 512):
                        kw = min(512, gw - k0)
                        d = g0 + k0
                        if r == 0:
                            if d < 2 * SEGDATA:
                                rhs = xt0a_sb[0:24]
                                c0 = _phys(d)
                            else:
                                rhs = xt0b_sb[0:24]
                                c0 = _phys(d) - 2 * SEGSTRIDE
                        else:
                            p0 = 32 * (r - 1)
                            rhs = xt12_sb[p0 : p0 + 24]
                            c0 = _phys(d)
                        nc.tensor.matmul(
                            out=ps[:, k0 : k0 + kw],
                            lhsT=w_ap,
                            rhs=rhs[:, c0 : c0 + kw],
                            start=True,
                            stop=True,
                        )
                    # relu(v - 2) -> bf16; ACT and DVE split the columns
                    sp = (ACT_COLS * gw) // GROUP
                    nc.scalar.activation(
                        o[:, g0 : g0 + sp],
                        ps[:, 0:sp],
                        relu,
                        bias=bias_sb,
                    )
                    nc.vector.tensor_scalar(
                        o[:, g0 + sp : g0 + gw],
                        ps[:, sp:gw],
                        -2.0,
                        0.0,
                        add,
                        mx,
                    )
                if r == 0:
                    # one full-phase store: pays for xt0's extra load DMA
                    # (8-lane budget) and overlaps phases 1-2 (phase-1 jumbo
                    # measured worse: it backs up the queue ahead of the
                    # phase-2 stores)
                    nc.sync.dma_start(out=y_d[0], in_=o[:])
                else:
                    # phase 2's first store fires a group earlier so the
                    # post-compute store queue is shorter (it is the tail)
                    sa = 9 * GROUP if r == 2 else STORE_A
                    nc.sync.dma_start(out=y_d[r, :, 0:sa], in_=o[:, 0:sa])
                    nc.sync.dma_start(out=y_d[r, :, sa:M], in_=o[:, sa:M])
    return nc


def _stage_inputs(x):
    """x: [8,1,L,4] f32 -> per-core {'xta': [56, M+128], 'xtb': [24, M]}."""
    w = _kmer_w()
    need = 12 * (2 * M - 1) + 28  # last col of the r=2 window reads up to here
    in_maps = []
    for b in range(x.shape[0]):
        xf = np.zeros(need, dtype=np.float32)
        xf[: L * 4] = x[b, 0].ravel()

        def band(r):
            out = np.empty((NROW, M), dtype=np.float32)
            for t in range(3):
                for s in range(4):
                    out[8 * t + s] = xf[4 * (r + t) + s :: 12][:M]
                    out[8 * t + s + 4] = xf[12 * M + 4 * (r + t) + s :: 12][:M]
            return out

        def segs(rows, data, wmat):
            """Pack data into NSEG padded segments (W leads segment 0)."""
            out = np.zeros((rows, NSEG, SEG), dtype=ml_dtypes.bfloat16)
            out[: wmat.shape[0], 0, 0:128] = wmat
            for k in range(NSEG):
                hi = SEGDATA * (k + 1) if k < NSEG - 1 else M
                chunk = data[:, SEGDATA * k : hi]
                off = 128 if k == 0 else 0
                out[: data.shape[0], k, off : off + chunk.shape[1]] = chunk
            return out

        b16 = ml_dtypes.bfloat16
        xt0 = segs(24, band(0).astype(b16), w)
        xt12 = np.zeros((56, NSEG, SEG), dtype=b16)
        xt12[0:24] = segs(24, band(1).astype(b16), w)
        xt12[32:56] = segs(24, band(2).astype(b16), w)
        in_maps.append(
            {
                "xt0a": np.ascontiguousarray(xt0[:, 0:2]),
                "xt0b": np.ascontiguousarray(xt0[:, 2:4]),
                "xt12": xt12,
            }
        )
    return in_maps


def _gather_output(results):
    out = np.empty((len(results), 1, 3 * Q, 64), dtype=np.float32)
    for b, res in enumerate(results):
        y = np.asarray(res["y"]).astype(np.float32)  # [3, 128, M]
        for r in range(3):
            zr = y[r].reshape(2, 64, M).transpose(0, 2, 1).reshape(2 * M, 64)
            out[b, 0, r * Q : (r + 1) * Q, :] = zr[:Q]
    return out


def _built_and_finalized():
    if "nc" not in _CACHE:
        nc = _build_bass()
        # run_bass_via_pjrt never finalizes; Bacc.finalize runs the register
        # allocation + sync-wait legalization passes walrus requires.
        nc.finalize()
        _CACHE["nc"] = nc
    return _CACHE["nc"]


def run(x, trace=False):
    nc = _built_and_finalized()
    in_maps = _stage_inputs(np.asarray(x, dtype=np.float32))
    bkr = run_bass_kernel_spmd(nc, in_maps, list(range(N_CORES)), trace=trace)
    return _gather_output(bkr.results), bkr


def kernel(x, W=None):
    out, _ = run(x, trace=False)
    return out


# revision 70
# speedup vs baseline: 1.0714x; 1.0377x over previous
"""Trainium2 Bass kernel for the k-mer transformer problem.

Semantics (k=3, one-hot 3-mer filters over 4 bases):
    z[l, c] = relu(x[l,d0] + x[l+1,d1] + x[l+2,d2] - 2)
      where c = 16*d0 + 4*d1 + d2,  l in [0, 99999)
    out[b, 0, r*33333 + q, c] = z[3q + r, c]      (mod-3 interleave)

Strategy: pure data parallel (batch elem b -> NeuronCore b), and the conv
is a matmul on the tensor engine (the only engine with headroom: vector
f32 adds cap at ~123 G elem/s, Pool's software tensor_tensor at ~40-65).

Key layout trick: within one phase r the positions l = 3q + r are stride-3,
so each output position consumes 12 *consecutive* x floats
x.flat[12q+4r : 12q+4r+12] -- no input replication. Two position streams
(A: q in [0,16704), B: q+16704) are packed as M=128 output rows
(64 channels x 2 streams), so the PE streams 2 positions per column.
The staged moving tensor holds one 24-row window per phase: row 8t+s of
phase r's window is x.flat[12m + 4(r+t) + s] for s<4 (stream A) and
x.flat[12(m+16704) + 4(r+t) + s-4] for s>=4 (stream B). Matmul operands
(both lhsT and rhs, which must share a base) must sit at SBUF base
partition 0/32/64, so the windows live at: xt0 [24 rows] with phase 0 at
rows 0:24, and xt12 [56 rows] with phase 1 at rows 0:24 and phase 2 at
rows 32:56. Phase 0's matmuls depend only on the small xt0 load (~1MB),
so the PE starts ~9us earlier than with a combined tile; xt12 streams in
under phase-0 compute. The stationary W[24,128] rides in the first 128
elements of every band's row segment 0 (one copy per base partition). W
is phase-invariant; the -2 bias folds into the relu evict, not the
matmul.

PSUM [128, 512] f32 banks (one matmul each, 2 banks per group, 4 groups
in flight so the PE never stalls) are evicted as relu(v-2) -> bf16 by ACT
and DVE in parallel column slices sized to measured rates (GPSIMD cannot
read PSUM). Output rides to HBM as bf16 (12.8 MB/core; harness tolerance
2e-2 dwarfs the ~6e-3 bf16 path error); the host transposes
[2*64, 16704] -> [q, c] and upcasts during the gather.

DMA budget: 2 loads + 2 stores x 3 phases = 8 <= 8 HWDGE sem lanes (9+
adds a lane-reuse wait -> walrus "Too many sync wait commands").

Measured on 8xTRN2 (neuron-profile): 114.0us baseline vector-engine
version -> 69.0us this version (PE starts at ~13us after the small xt0a
load and runs 44us busy at an observed sustained ~1.2GHz, 1 col/cycle;
stores + a ~6.5us semaphore-drain epilogue make up the tail).
"""

import sys

import ml_dtypes
import numpy as np

sys.path.insert(0, "/opt/trn_rl_repo")

import concourse.bacc as bacc  # noqa: E402
import concourse.mybir as mybir  # noqa: E402
from concourse.bass_utils import run_bass_kernel_spmd  # noqa: E402
from concourse.tile import TileContext  # noqa: E402

P = 128
Q = 33333  # valid q-positions per phase (99999 / 3)
M = 16704  # columns per stream (2 streams: q and q + M; 2*M = 33408 >= Q)
NROW = 24  # XT rows: 3 t-blocks x (4 A-rows + 4 B-rows), one window/phase
GROUP = 1024  # psum group columns (2 banks x 512); 4 groups in flight
# evict column split per group, proportional to measured engine rates
# (GPSIMD/Pool cannot read PSUM; ACT ~1.03 GHz and DVE ~0.89 GHz effective)
ACT_COLS = 608  # DVE gets the rest (416)
STORE_A = 10 * GROUP  # uneven store split so the trailing store is short
# Input loads run at half DMA-engine rate for descriptors above ~8KB, and
# in-place splits coalesce back into one big descriptor. So each staged row
# is six 3200-element segments separated by 64-element SBUF gaps
# (non-adjacent -> no coalescing; ~6.4KB descriptors run at full rate).
# Segment k holds data columns [3072k, 3072(k+1)) (last: ..16704), with W
# in the first 128 elements of segment 0 (data starts at offset 128 there).
# 4096 is a psum-group multiple, so no matmul slice crosses a segment
# boundary. Both loads ride the SP HWDGE ring: measured alternatives were
# slower (SWDGE/GPSIMD loads: +8us; xtb on the ACT ring: +19us, it stalls
# the ACT evict stream).
SEGDATA = 4096
SEG = 4416
SEGSTRIDE = SEG + 64  # 4480
NSEG = 4
ROWW = NSEG * SEGSTRIDE  # 17920 elements per staged SBUF row


def _phys(d):
    """Physical column of logical data column d."""
    k = min(d // SEGDATA, NSEG - 1)
    return k * SEGSTRIDE + (128 if k == 0 else 0) + (d - SEGDATA * k)
L = 100001
N_CORES = 8

_CACHE = {}


def _kmer_w():
    """Stationary [24, 128] weights: row 8*jj+s, s<4 -> tap (jj, d=s) of
    stream A (cols 0:64), s>=4 -> tap (jj, d=s-4) of stream B (cols 64:128).
    """
    w = np.zeros((24, 128), dtype=np.float32)
    c = np.arange(64)
    digits = np.stack([c // 16, (c // 4) % 4, c % 4])  # [jj, c]
    for k in range(24):
        jj, s = k // 8, k % 8
        blk, d = (0, s) if s < 4 else (1, s - 4)
        w[k, 64 * blk + c[digits[jj] == d]] = 1.0
    return w.astype(ml_dtypes.bfloat16)


def _build_bass():
    # Bacc (not raw Bass): its finalize() runs generate_event_semaphores,
    # which splits multi-sem waits (HW allows at most 1 wait per inst).
    nc = bacc.Bacc()
    f32 = mybir.dt.float32
    bf16 = mybir.dt.bfloat16
    add = mybir.AluOpType.add
    mx = mybir.AluOpType.max
    relu = mybir.ActivationFunctionType.Relu

    xt0a_d = nc.declare_dram_parameter("xt0a", [24, 2, SEG], bf16, isOutput=False)
    xt0b_d = nc.declare_dram_parameter("xt0b", [24, 2, SEG], bf16, isOutput=False)
    xt12_d = nc.declare_dram_parameter("xt12", [56, NSEG, SEG], bf16, isOutput=False)
    y_d = nc.declare_dram_parameter("y", [3, P, M], bf16, isOutput=True)

    with TileContext(nc) as tc:
        with (
            tc.tile_pool(name="xp", bufs=1) as xp,
            tc.tile_pool(name="pp", bufs=4, space="PSUM") as pp,
            tc.tile_pool(name="op_", bufs=2) as op_,
        ):
            # phase 0's band rides two half-tiles so the PE can start after
            # ~0.4MB of load; its matmuls for cols >= 8192 take lhsT from
            # xt0a (cross-tile, same base partition 0)
            xt0a_sb = xp.tile([24, 2 * SEGSTRIDE], bf16)
            nc.sync.dma_start(
                out=xt0a_sb.rearrange("p (c g) -> p c g", c=2)[:, :, 0:SEG],
                in_=xt0a_d[:],
            )
            xt0b_sb = xp.tile([24, 2 * SEGSTRIDE], bf16, tag="xt0b")
            nc.sync.dma_start(
                out=xt0b_sb.rearrange("p (c g) -> p c g", c=2)[:, :, 0:SEG],
                in_=xt0b_d[:],
            )
            xt12_sb = xp.tile([56, ROWW], bf16, tag="xt12")
            nc.sync.dma_start(
                out=xt12_sb.rearrange("p (c g) -> p c g", c=NSEG)[:, :, 0:SEG],
                in_=xt12_d[:],
            )
            bias_sb = xp.tile([P, 1], f32, tag="bias")
            nc.vector.memset(bias_sb, -2.0)
            for r in range(3):
                o = op_.tile([P, M], bf16, tag="o")
                # lhsT and rhs must share a base partition (0/32/64)
                if r == 0:
                    w_ap = xt0a_sb[0:24, 0:128]
                else:
                    w_ap = xt12_sb[32 * (r - 1) : 32 * (r - 1) + 24, 0:128]
                for g0 in range(0, M, GROUP):
                    gw = min(GROUP, M - g0)
                    ps = pp.tile([P, GROUP], f32, tag="ps")
                    for k0 in range(0, gw, 512):
                        kw = min(512, gw - k0)
                        d = g0 + k0
                        if r == 0:
                            if d < 2 * SEGDATA:
                                rhs = xt0a_sb[0:24]
                                c0 = _phys(d)
                            else:
                                rhs = xt0b_sb[0:24]
                                c0 = _phys(d) - 2 * SEGSTRIDE
                        else:
                            p0 = 32 * (r - 1)
                            rhs = xt12_sb[p0 : p0 + 24]
                            c0 = _phys(d)
                        nc.tensor.matmul(
                            out=ps[:, k0 : k0 + kw],
                            lhsT=w_ap,
                            rhs=rhs[:, c0 : c0 + kw],
                            start=True,
                            stop=True,
                        )
                    # relu(v - 2) -> bf16; whole-group evicts alternate
                    # between ACT and DVE (halves instruction/sem count vs
                    # splitting every group; both stay under the PE's 44us)
                    if (g0 // GROUP) % 2 == 0:
                        nc.scalar.activation(
                            o[:, g0 : g0 + gw],
                            ps[:, 0:gw],
                            relu,
                            bias=bias_sb,
                        )
                    else:
                        nc.vector.tensor_scalar(
                            o[:, g0 : g0 + gw],
                            ps[:, 0:gw],
                            -2.0,
                            0.0,
                            add,
                            mx,
                        )
                if r == 0:
                    # one full-phase store: pays for xt0's extra load DMA
                    # (8-lane budget) and overlaps phases 1-2 (phase-1 jumbo
                    # measured worse: it backs up the queue ahead of the
                    # phase-2 stores)
                    nc.sync.dma_start(out=y_d[0], in_=o[:])
                else:
                    # phase 2's first store fires a group earlier so the
                    # post-compute store queue is shorter (it is the tail)
                    sa = 9 * GROUP if r == 2 else STORE_A
                    nc.sync.dma_start(out=y_d[r, :, 0:sa], in_=o[:, 0:sa])
                    nc.sync.dma_start(out=y_d[r, :, sa:M], in_=o[:, sa:M])
    return nc


def _stage_inputs(x):
    """x: [8,1,L,4] f32 -> per-core {'xta': [56, M+128], 'xtb': [24, M]}."""
    w = _kmer_w()
    need = 12 * (2 * M - 1) + 28  # last col of the r=2 window reads up to here
    in_maps = []
    for b in range(x.shape[0]):
        xf = np.zeros(need, dtype=np.float32)
        xf[: L * 4] = x[b, 0].ravel()

        def band(r):
            out = np.empty((NROW, M), dtype=np.float32)
            for t in range(3):
                for s in range(4):
                    out[8 * t + s] = xf[4 * (r + t) + s :: 12][:M]
                    out[8 * t + s + 4] = xf[12 * M + 4 * (r + t) + s :: 12][:M]
            return out

        def segs(rows, data, wmat):
            """Pack data into NSEG padded segments (W leads segment 0)."""
            out = np.zeros((rows, NSEG, SEG), dtype=ml_dtypes.bfloat16)
            out[: wmat.shape[0], 0, 0:128] = wmat
            for k in range(NSEG):
                hi = SEGDATA * (k + 1) if k < NSEG - 1 else M
                chunk = data[:, SEGDATA * k : hi]
                off = 128 if k == 0 else 0
                out[: data.shape[0], k, off : off + chunk.shape[1]] = chunk
            return out

        b16 = ml_dtypes.bfloat16
        xt0 = segs(24, band(0).astype(b16), w)
        xt12 = np.zeros((56, NSEG, SEG), dtype=b16)
        xt12[0:24] = segs(24, band(1).astype(b16), w)
        xt12[32:56] = segs(24, band(2).astype(b16), w)
        in_maps.append(
            {
                "xt0a": np.ascontiguousarray(xt0[:, 0:2]),
                "xt0b": np.ascontiguousarray(xt0[:, 2:4]),
                "xt12": xt12,
            }
        )
    return in_maps


def _gather_output(results):
    out = np.empty((len(results), 1, 3 * Q, 64), dtype=np.float32)
    for b, res in enumerate(results):
        y = np.asarray(res["y"]).astype(np.float32)  # [3, 128, M]
        for r in range(3):
            zr = y[r].reshape(2, 64, M).transpose(0, 2, 1).reshape(2 * M, 64)
            out[b, 0, r * Q : (r + 1) * Q, :] = zr[:Q]
    return out


def _built_and_finalized():
    if "nc" not in _CACHE:
        nc = _build_bass()
        # run_bass_via_pjrt never finalizes; Bacc.finalize runs the register
        # allocation + sync-wait legalization passes walrus requires.
        nc.finalize()
        _CACHE["nc"] = nc
    return _CACHE["nc"]


def run(x, trace=False):
    nc = _built_and_finalized()
    in_maps = _stage_inputs(np.asarray(x, dtype=np.float32))
    bkr = run_bass_kernel_spmd(nc, in_maps, list(range(N_CORES)), trace=trace)
    return _gather_output(bkr.results), bkr


def kernel(x, W=None):
    out, _ = run(x, trace=False)
    return out
